# revision 3
# baseline (speedup 1.0000x reference)
"""AUGRU cell (attention-scaled GRU update) on 8 Trainium2 NeuronCores.

Data-parallel: batch B=65536 sharded 8 ways (8192 rows/core); gate weights
replicated.  Per core (gate-major layout, batch on the free axis):

  gates_x = x @ W_x.T + b_x
  gates_h = h @ W_h.T + b_h
  u = sigmoid(U); r = sigmoid(R); t = tanh(Cx + r*Ch)
  h_new = h + att*u*(t - h)

v11 design -- 7 matmuls/group (was 9), ACT-bias sigmoids, pair-wide ops:
  - biases enter via the ACT bias operand (per-partition [P,1]), killing the
    two K=1 bias-prefill matmuls per group of v10.  PSUM pur tile is laid
    out [U_g0, U_g1, R_g0, R_g1] so sigU reads banks 0-1 as one contiguous
    [P, 1024] op and sigR banks 2-3 (one ACT op per gate per PAIR).
  - identity matmul still merges m = (Ch+bCh)*r into the open Cx bank
    (216ns on PE beats ~1.2us on DVE); tanh reads PSUM with bCx bias and is
    emitted one pipeline stage late (never heads a stalled ACT queue).
  - PSUM budget: pur pair tile 4 banks (bufs=1) + pcx group (bufs=2) +
    pch group (bufs=2) = 8 banks exactly.
  - x/h/attb live in single [P, 8192] SBUF tiles; DMAs fill column ranges
    (256K/256K/512K/1M slices so compute starts after ~0.9us of wire);
    range-based deps replace pool rotation.
  - epilogue per pair (1024 cols): ua=att*u (DVE), d=t-h (GPSIMD),
    q=ua*d (DVE), ho=h+q (DVE), one output DMA; last pair split per group
    across DVE to cut the tail.
"""

import sys

sys.path.insert(0, "/opt/trn_rl_repo")

from contextlib import ExitStack

import numpy as np
import ml_dtypes

import concourse.bass as bass
import concourse.tile as tile
from concourse import bacc, mybir
from concourse.bass_utils import run_bass_kernel_spmd

F32 = mybir.dt.float32
BF16 = mybir.dt.bfloat16
AF = mybir.ActivationFunctionType
OP = mybir.AluOpType
BFNP = ml_dtypes.bfloat16

B = 65536
NCORES = 8
BL = B // NCORES  # 8192 rows per core
I = 128
H = 128
P = 128
ROWS = 512  # batch rows per group (one fp32 PSUM bank per gate)
NGROUPS = BL // ROWS  # 16
NP = NGROUPS // 2  # 8 pairs
PR = 2 * ROWS  # pair width 1024


def build_program():
    nc = bacc.Bacc("TRN2", target_bir_lowering=False, debug=False)

    xT_d = nc.dram_tensor("xT", [I, BL], BF16, kind="ExternalInput").ap()
    hT_d = nc.dram_tensor("hT", [H, BL], BF16, kind="ExternalInput").ap()
    ab_d = nc.dram_tensor("attb", [P, BL], BF16, kind="ExternalInput").ap()
    wx_d = nc.dram_tensor("wxT", [I, 3, P], BF16, kind="ExternalInput").ap()
    wh_d = nc.dram_tensor("whT", [H, 3, P], BF16, kind="ExternalInput").ap()
    bc_d = nc.dram_tensor("bcol", [P, 4], F32, kind="ExternalInput").ap()
    id_d = nc.dram_tensor("ident", [P, P], BF16, kind="ExternalInput").ap()
    o_d = nc.dram_tensor("h_newT", [H, BL], BF16, kind="ExternalOutput").ap()

    with tile.TileContext(nc) as tc, ExitStack() as ctx:
        consts = ctx.enter_context(tc.tile_pool(name="consts", bufs=1))
        io = ctx.enter_context(tc.tile_pool(name="io", bufs=1))
        gp = ctx.enter_context(tc.tile_pool(name="gp", bufs=2))
        ep = ctx.enter_context(tc.tile_pool(name="ep", bufs=3))
        pur = ctx.enter_context(tc.tile_pool(name="pur", bufs=1, space="PSUM"))
        pcx = ctx.enter_context(tc.tile_pool(name="pcx", bufs=2, space="PSUM"))
        pch = ctx.enter_context(tc.tile_pool(name="pch", bufs=2, space="PSUM"))

        # ---------------- one-time setup ----------------
        wT = consts.tile([P, 6, P], BF16, tag="wT")  # [xu, xr, xc, hu, hr, hc]
        nc.sync.dma_start(wT[:, 0:3, :], wx_d)
        nc.sync.dma_start(wT[:, 3:6, :], wh_d)
        bcol = consts.tile([P, 4], F32, tag="bcol")  # [bU, bR, bCx, bCh]
        nc.sync.dma_start(bcol, bc_d)
        ident = consts.tile([P, P], BF16, tag="ident")
        nc.sync.dma_start(ident, id_d)

        # whole-core input/attb tiles; DMAs fill column ranges
        xs = io.tile([P, BL], BF16, tag="xs")
        hs = io.tile([P, BL], BF16, tag="hs")
        ab = io.tile([P, BL], BF16, tag="ab")
        # slice boundaries: fast pipeline start, then big efficient chunks
        CUTS = [0, PR, 2 * PR, 4 * PR, 8 * PR]
        for lo, hi in zip(CUTS[:-1], CUTS[1:]):
            nc.sync.dma_start(xs[:, lo:hi], xT_d[:, lo:hi])
            nc.sync.dma_start(hs[:, lo:hi], hT_d[:, lo:hi])
            nc.sync.dma_start(ab[:, lo:hi], ab_d[:, lo:hi])

        stB = [None] * NP  # (ur, cx0, cx1, ch0, ch1) per pair
        ups = [None] * NP  # u pair tiles [P, 2, ROWS] bf16
        rps = [None] * NP
        mps = [None] * NP
        tps = [None] * NP
        uas = [None] * NP

        def stage_b(p):
            sl0 = slice(2 * p * ROWS, (2 * p + 1) * ROWS)
            sl1 = slice((2 * p + 1) * ROWS, (2 * p + 2) * ROWS)
            xg = (xs[:, sl0], xs[:, sl1])
            hg = (hs[:, sl0], hs[:, sl1])
            # pur banks: [U_g0, U_g1, R_g0, R_g1] -> sigU reads [:,0,:,:]
            ur = pur.tile([P, 2, 2, ROWS], F32, tag="ur")
            # grouped by stationary weight so LDWEIGHTS fires once per weight
            nc.tensor.matmul(ur[:, 0, 0, :], lhsT=wT[:, 0, :], rhs=xg[0], start=True, stop=False)
            nc.tensor.matmul(ur[:, 0, 1, :], lhsT=wT[:, 0, :], rhs=xg[1], start=True, stop=False)
            nc.tensor.matmul(ur[:, 1, 0, :], lhsT=wT[:, 1, :], rhs=xg[0], start=True, stop=False)
            nc.tensor.matmul(ur[:, 1, 1, :], lhsT=wT[:, 1, :], rhs=xg[1], start=True, stop=False)
            nc.tensor.matmul(ur[:, 0, 0, :], lhsT=wT[:, 3, :], rhs=hg[0], start=False, stop=True)
            nc.tensor.matmul(ur[:, 0, 1, :], lhsT=wT[:, 3, :], rhs=hg[1], start=False, stop=True)
            nc.tensor.matmul(ur[:, 1, 0, :], lhsT=wT[:, 4, :], rhs=hg[0], start=False, stop=True)
            nc.tensor.matmul(ur[:, 1, 1, :], lhsT=wT[:, 4, :], rhs=hg[1], start=False, stop=True)
            cx0 = pcx.tile([P, ROWS], F32, tag="cx")
            cx1 = pcx.tile([P, ROWS], F32, tag="cx")
            ch0 = pch.tile([P, ROWS], F32, tag="ch")
            ch1 = pch.tile([P, ROWS], F32, tag="ch")
            nc.tensor.matmul(cx0, lhsT=wT[:, 2, :], rhs=xg[0], start=True, stop=False)  # stays open
            nc.tensor.matmul(cx1, lhsT=wT[:, 2, :], rhs=xg[1], start=True, stop=False)
            nc.tensor.matmul(ch0, lhsT=wT[:, 5, :], rhs=hg[0], start=True, stop=True)
            nc.tensor.matmul(ch1, lhsT=wT[:, 5, :], rhs=hg[1], start=True, stop=True)
            stB[p] = (ur, cx0, cx1, ch0, ch1)

        def stage_c(p):
            ur, cx0, cx1, ch0, ch1 = stB[p]
            u = gp.tile([P, 2, ROWS], BF16, tag="u")
            r = gp.tile([P, 2, ROWS], BF16, tag="r")
            nc.scalar.activation(u, ur[:, 0, :, :], AF.Sigmoid, bias=bcol[:, 0:1])
            nc.scalar.activation(r, ur[:, 1, :, :], AF.Sigmoid, bias=bcol[:, 1:2])
            m = gp.tile([P, 2, ROWS], BF16, tag="m")
            nc.vector.scalar_tensor_tensor(
                m[:, 0, :], in0=ch0, scalar=bcol[:, 3:4], in1=r[:, 0, :],
                op0=OP.add, op1=OP.mult,
            )
            nc.vector.scalar_tensor_tensor(
                m[:, 1, :], in0=ch1, scalar=bcol[:, 3:4], in1=r[:, 1, :],
                op0=OP.add, op1=OP.mult,
            )
            nc.tensor.matmul(cx0, lhsT=ident, rhs=m[:, 0, :], start=False, stop=True)
            nc.tensor.matmul(cx1, lhsT=ident, rhs=m[:, 1, :], start=False, stop=True)
            ua = gp.tile([P, PR], BF16, tag="ua")
            nc.vector.tensor_tensor(ua, u.rearrange("p a b -> p (a b)"), ab[:, 2 * p * ROWS : 2 * p * ROWS + PR], OP.mult)
            ups[p], rps[p], mps[p], uas[p] = u, r, m, ua

        def stage_t(p):
            # tanh emitted one stage late: never heads the ACT queue stalled
            ur, cx0, cx1, ch0, ch1 = stB[p]
            t = gp.tile([P, 2, ROWS], BF16, tag="t")
            nc.scalar.activation(t[:, 0, :], cx0, AF.Tanh, bias=bcol[:, 2:3])
            nc.scalar.activation(t[:, 1, :], cx1, AF.Tanh, bias=bcol[:, 2:3])
            tps[p] = t

        def stage_e(p):
            base = 2 * p * ROWS
            hsl = hs[:, base : base + PR]
            t, ua = tps[p], uas[p]
            d = ep.tile([P, PR], BF16, tag="d")
            q = ep.tile([P, PR], BF16, tag="q")
            ho = ep.tile([P, PR], BF16, tag="ho")
            if p == NP - 1:
                # last pair: split per group across engines to cut the tail
                for g in range(2):
                    gsl = slice(g * ROWS, (g + 1) * ROWS)
                    eng = nc.gpsimd if g == 0 else nc.vector
                    eng.tensor_tensor(d[:, gsl], t[:, g, :], hs[:, base + g * ROWS : base + (g + 1) * ROWS], OP.subtract)
                    nc.vector.tensor_tensor(q[:, gsl], d[:, gsl], ua[:, gsl], OP.mult)
                    nc.vector.tensor_tensor(ho[:, gsl], q[:, gsl], hs[:, base + g * ROWS : base + (g + 1) * ROWS], OP.add)
                    nc.sync.dma_start(o_d[:, base + g * ROWS : base + (g + 1) * ROWS], ho[:, gsl])
                return
            nc.gpsimd.tensor_tensor(d, t.rearrange("p a b -> p (a b)"), hsl, OP.subtract)
            nc.vector.tensor_tensor(q, d, ua, OP.mult)
            nc.vector.tensor_tensor(ho, q, hsl, OP.add)
            nc.sync.dma_start(o_d[:, base : base + PR], ho)

        for k in range(NP + 3):
            if k < NP:
                stage_b(k)
            if 1 <= k < NP + 1:
                stage_c(k - 1)
            if 2 <= k < NP + 2:
                stage_t(k - 2)
            if 3 <= k < NP + 3:
                stage_e(k - 3)

    nc.compile()
    return nc


_NC_CACHE = []


def _get_nc():
    if not _NC_CACHE:
        _NC_CACHE.append(build_program())
    return _NC_CACHE[0]


def make_in_maps(x, h_prev, att_score, W_x, b_x, W_h, b_h):
    """Shard + stage inputs for the 8 cores (bf16 wire format)."""
    x = np.asarray(x, dtype=np.float32)
    h_prev = np.asarray(h_prev, dtype=np.float32)
    att = np.asarray(att_score, dtype=np.float32)
    W_x = np.asarray(W_x, dtype=np.float32)
    W_h = np.asarray(W_h, dtype=np.float32)
    b_x = np.asarray(b_x, dtype=np.float32)
    b_h = np.asarray(b_h, dtype=np.float32)

    wxT = np.ascontiguousarray(W_x.T.reshape(I, 3, P).astype(BFNP))
    whT = np.ascontiguousarray(W_h.T.reshape(H, 3, P).astype(BFNP))
    bsum = b_x + b_h  # valid for U and R blocks
    bcol = np.stack(
        [bsum[0:P], bsum[P : 2 * P], b_x[2 * P : 3 * P], b_h[2 * P : 3 * P]], axis=1
    ).astype(np.float32)
    ident = np.eye(P, dtype=BFNP)

    in_maps = []
    for c in range(NCORES):
        s = slice(c * BL, (c + 1) * BL)
        attb = np.broadcast_to(att[s].astype(BFNP), (P, BL))
        in_maps.append(
            {
                "xT": np.ascontiguousarray(x[s].T.astype(BFNP)),
                "hT": np.ascontiguousarray(h_prev[s].T.astype(BFNP)),
                "attb": np.ascontiguousarray(attb),
                "wxT": wxT,
                "whT": whT,
                "bcol": bcol,
                "ident": ident,
            }
        )
    return in_maps


def kernel(x, h_prev, att_score, W_x, b_x, W_h, b_h, **_unused):
    nc = _get_nc()
    in_maps = make_in_maps(x, h_prev, att_score, W_x, b_x, W_h, b_h)
    res = run_bass_kernel_spmd(nc, in_maps, list(range(NCORES)))
    out = np.concatenate(
        [
            np.asarray(res.results[c]["h_newT"]).astype(np.float32).T
            for c in range(NCORES)
        ],
        axis=0,
    )
    return np.ascontiguousarray(out)
